# revision 16
# baseline (speedup 1.0000x reference)
"""Trainium2 Bass kernel for nn_BasicNCAModel — fp8 DoubleRow mm1 variant.

Same structure as the f16 kernel (unified stack, carried mm2, fire trick)
with mm1 in fp8e4: the dy=-1 and dy=+1 taps fuse into ONE DoubleRow matmul
(virtual K=256, 2 multiplies/cycle), the center tap is a normal fp8 matmul,
so mm1 is 2 matmuls per (j,g) instead of 3. The stack is fp8 pitch-128
(DoubleRow needs a single-stride moving AP: k-tile pair stride 256 B), fed
from a pitch-130 fp8 shadow x8 whose wrap cols give the dx=+-1 shifts their
circular reads. Weights are scaled by SA=32 to sit in fp8e4's normal range
(fire weight 240 = max normal; effective M = 240/32 = 7.5 >> |h+b1|), and
the 1/SA is folded into W2 (f16), so no extra scaling ops anywhere.
"""

import sys

if "/opt/trn_rl_repo" not in sys.path:
    sys.path.insert(0, "/opt/trn_rl_repo")

import numpy as np
import ml_dtypes

F8 = ml_dtypes.float8_e4m3

C = 24
NIC = 4
H = 128
WID = 128
HID = 128
STEPS = 8
B = 8
G = 4
RG = 32
PITCH = 132    # f16 master pitch
GROWS = 34
FB = GROWS * PITCH
SP8 = 128      # fp8 stack pitch (contiguous pixel rows)
SROWS = 130
SFL8 = SROWS * SP8
X8P = 130      # fp8 shadow pitch (wrap col + 128 + wrap col)
X8L = 32 * X8P
TW = 512
SA = 32.0      # fp8 weight scale
MF8 = 240.0    # fire weight (fp8e4 max normal); effective M = MF8/SA
SC_FD = 416

_CACHE = {}


def _build_module():
    from concourse import bacc, mybir, tile
    from concourse.ap import AP

    f32 = mybir.dt.float32
    f16 = mybir.dt.float16
    f8 = mybir.dt.float8e4
    Alu = mybir.AluOpType
    Act = mybir.ActivationFunctionType
    DR = mybir.MatmulPerfMode.DoubleRow

    nc = bacc.Bacc(
        "TRN2",
        target_bir_lowering=False,
        debug=False,
        enable_asserts=False,
        num_devices=8,
    )

    apdr = nc.dram_tensor("apdr", [128, 256], f8, kind="ExternalInput").ap()
    apc = nc.dram_tensor("apc", [128, 128], f8, kind="ExternalInput").ap()
    w2p = nc.dram_tensor("w2p", [128, 32], f16, kind="ExternalInput").ap()
    b1col = nc.dram_tensor("b1col", [128, 1], f32, kind="ExternalInput").ap()
    x16in = nc.dram_tensor("x16in", [128, FB], f16, kind="ExternalInput").ap()
    x8in = nc.dram_tensor("x8in", [128, X8L], f8, kind="ExternalInput").ap()
    firein = nc.dram_tensor("firein", [128, 4096], f8, kind="ExternalInput").ap()
    stkin = nc.dram_tensor("stkin", [128, SFL8], f8, kind="ExternalInput").ap()
    xout = nc.dram_tensor("xout", [128, 4096], f32, kind="ExternalOutput").ap()

    with tile.TileContext(nc) as tc:
        import contextlib

        with contextlib.ExitStack() as ctx:
            sing = ctx.enter_context(tc.tile_pool(name="sing", bufs=1))
            hpool = ctx.enter_context(tc.tile_pool(name="h", bufs=6, space="PSUM"))
            dxpool = ctx.enter_context(tc.tile_pool(name="dx", bufs=2, space="PSUM"))
            hsb = ctx.enter_context(tc.tile_pool(name="hsb", bufs=8))

            x16a = sing.tile([128, FB], f16)
            x16b = sing.tile([128, FB], f16)
            x8 = sing.tile([128, X8L], f8)
            xof = sing.tile([128, 4096], f32)
            fire = sing.tile([128, 4096], f8)
            A8 = sing.tile([128, 256], f8)
            AC = sing.tile([128, 128], f8)
            W2s = sing.tile([128, 32], f16)
            zeros = sing.tile([128, TW - SC_FD], f32)
            b1c = sing.tile([128, 1], f32)
            stk = [sing.tile([128, SFL8], f8, name=f"stk_{b}") for b in range(2)]

            nc.scalar.dma_start(A8[:], apdr[:])
            nc.scalar.dma_start(AC[:], apc[:])
            QS = [nc.sync, nc.gpsimd, nc.sync, nc.scalar]
            for g in range(G):
                lo, hi = (32 * g + 4) * SP8, (32 * g + 22) * SP8
                QS[g].dma_start(stk[0][:, lo:hi], stkin[:, lo:hi])
            gaps = [(0, 4), (22, 36), (54, 68), (86, 100), (118, 130)]
            for k, (a, b) in enumerate(gaps):
                QS[k % 3].dma_start(
                    stk[0][:, a * SP8 : b * SP8],
                    stkin[:, a * SP8 : b * SP8],
                )
            nc.gpsimd.dma_start(W2s[:], w2p[:])
            nc.gpsimd.dma_start(b1c[:], b1col[:])
            nc.sync.dma_start(x16a[:], x16in[:])
            nc.sync.dma_start(x8[:], x8in[:])
            nc.sync.dma_start(fire[:], firein[:])
            nc.gpsimd.memset(stk[1][64:128, :], 0.0)
            nc.gpsimd.memset(zeros[:], 0.0)

            xf16 = [x16a, x16b]
            fire3 = fire[:].rearrange("p (r w) -> p r w", w=128)
            xo3 = xof[:].rearrange("p (r w) -> p r w", w=128)
            x8v = x8[:].rearrange("p (r w) -> p r w", w=X8P)
            a8v = A8[:].rearrange("p (t n) -> p t n", t=2)
            SLICES = [(0, 4), (4, 16), (16, 24), (24, 32)]
            JORD = [2, 3, 1, 4, 5, 0, 6, 7]
            TRIGGER = {0: 0, 1: 1, 5: 2, 7: 3}

            def emit_slice(s, sl):
                sb = stk[(s + 1) % 2]
                la, lb = SLICES[sl]
                for g in range(G):
                    dlo = (32 * g + 1 + la) * SP8
                    dhi = (32 * g + 1 + lb) * SP8
                    for d in range(3):
                        eng = [nc.sync, nc.gpsimd][(g + d) % 2]
                        eng.dma_start(
                            sb[24 * d : 24 * d + 24, dlo:dhi],
                            x8v[32 * g : 32 * g + 24, la:lb, d : d + 128],
                        )

            def emit_fire(s):
                sb = (s + 1) % 2
                for g in range(G):
                    nc.gpsimd.dma_start(
                        stk[sb][72:73, (32 * g + 1) * SP8 : (32 * g + 33) * SP8],
                        fire3[32 * g + s + 1 : 32 * g + s + 2, :, :],
                    )

            state = {"prev": None}

            def mm2_update(s, j, hss, last):
                r0 = 4 * j + 1
                xc = xf16[s % 2][:].rearrange("p (r w) -> p r w", w=PITCH)
                xn = xf16[(s + 1) % 2][:].rearrange("p (r w) -> p r w", w=PITCH)
                dxt = dxpool.tile([128, TW], f32, tag="dx", name=f"dx_{s}_{j}")
                for g in range(G):
                    nc.tensor.matmul(
                        dxt[32 * g : 32 * g + 32, :],
                        W2s[:],
                        hss[g][:],
                        start=True,
                        stop=True,
                        tile_position=(0, 32 * g),
                    )
                dx3 = dxt[:].rearrange("p (a b) -> p a b", b=128)
                if last:
                    nc.vector.tensor_tensor(
                        xo3[:, 4 * j : 4 * j + 4, :],
                        dx3,
                        xc[:, r0 : r0 + 4, 1:129],
                        Alu.add,
                    )
                    return
                nc.vector.tensor_tensor(
                    xn[:, r0 : r0 + 4, 1:129],
                    dx3,
                    xc[:, r0 : r0 + 4, 1:129],
                    Alu.add,
                )
                nc.vector.tensor_copy(
                    xn[:, r0 : r0 + 4, 0:1], xn[:, r0 : r0 + 4, 128:129]
                )
                nc.vector.tensor_copy(
                    xn[:, r0 : r0 + 4, 129:130], xn[:, r0 : r0 + 4, 1:2]
                )
                # fp8 shadow of the updated rows (feeds next stack build)
                nc.vector.tensor_copy(
                    x8v[:, 4 * j : 4 * j + 4, 0:130],
                    xn[:, r0 : r0 + 4, 0:130],
                )
                if j in TRIGGER:
                    emit_slice(s, TRIGGER[j])
                    nb = stk[(s + 1) % 2]
                    if TRIGGER[j] == 0:
                        # halo srow 129 (image row 0) <- srow 1
                        nc.gpsimd.tensor_copy(
                            nb[:80, 129 * SP8 : 130 * SP8],
                            nb[:80, 1 * SP8 : 2 * SP8],
                        )
                    if TRIGGER[j] == 3:
                        # halo srow 0 (image row 127) <- srow 128
                        nc.gpsimd.tensor_copy(
                            nb[:80, 0:SP8],
                            nb[:80, 128 * SP8 : 129 * SP8],
                        )

            for s in range(STEPS):
                last = s + 1 == STEPS
                if not last:
                    emit_fire(s)
                sT = stk[s % 2][:]
                for j in JORD:
                    hss = []
                    for g in range(G):
                        sr0 = 32 * g + 4 * j + 1
                        ht = hpool.tile(
                            [128, TW], f32, tag="h", name=f"h_{s}_{j}_{g}"
                        )
                        # DoubleRow: k-tile 0 = dy=-1 rows, k-tile 1 =
                        # dy=+1 rows (pair stride 2 srows = 256 B)
                        rhs_dr = AP(
                            tensor=sT.tensor,
                            offset=sT.offset + (sr0 - 1) * SP8,
                            ap=[[SFL8, 128], [2 * SP8, 2], [1, 512]],
                        )
                        nc.tensor.matmul(
                            ht[:, :],
                            a8v,
                            rhs_dr,
                            start=True,
                            stop=False,
                            perf_mode=DR,
                            tile_position=(0, 0),
                        )
                        rhs_c = sT[:, sr0 * SP8 : sr0 * SP8 + TW]
                        nc.tensor.matmul(
                            ht[:, :],
                            AC[:],
                            rhs_c,
                            start=False,
                            stop=True,
                            tile_position=(0, 0),
                        )
                        hs = hsb.tile(
                            [128, TW], f16, tag="hsb", name=f"hs_{s}_{j}_{g}"
                        )
                        nc.scalar.activation(
                            hs[:, :SC_FD],
                            ht[:, :SC_FD],
                            Act.Relu,
                            bias=b1c[:],
                        )
                        nc.vector.scalar_tensor_tensor(
                            hs[:, SC_FD:],
                            ht[:, SC_FD:],
                            b1c[:],
                            zeros[:],
                            Alu.add,
                            Alu.max,
                        )
                        hss.append(hs)
                    if state["prev"] is not None:
                        mm2_update(*state["prev"])
                    state["prev"] = (s, j, hss, last)
            mm2_update(*state["prev"])

            nc.sync.dma_start(xout[:, :], xof[:, :])

    nc.compile()
    return nc


def _get_module():
    if "nc" not in _CACHE:
        _CACHE["nc"] = _build_module()
    return _CACHE["nc"]


def _prep_weights(w1, w2, W1, b1, W2):
    A = np.zeros((9, HID, C), np.float32)
    for t in range(9):
        dy, dxx = t // 3 - 1, t % 3 - 1
        A[t] = (
            W1[:, 24:48] * w1[dy + 1, dxx + 1, 0][None, :]
            + W1[:, 48:72] * w2[dy + 1, dxx + 1, 0][None, :]
        )
    A[4] += W1[:, :24]
    apdr = np.zeros((128, 256), np.float32)
    apc = np.zeros((128, 128), np.float32)
    for d in range(3):
        rows = slice(24 * d, 24 * d + 24)
        apdr[rows, 0:128] = SA * A[0 * 3 + d].T      # dy = -1
        apdr[rows, 128:256] = SA * A[2 * 3 + d].T    # dy = +1
        apc[rows, :] = SA * A[1 * 3 + d].T           # dy = 0
    apc[72, :] = MF8
    w2pk = np.zeros((128, 32), np.float32)
    w2pk[:, NIC:C] = W2[NIC:C].T / SA
    b1c = (SA * b1 - MF8).reshape(128, 1).astype(np.float32)
    return apdr.astype(F8), apc.astype(F8), w2pk.astype(np.float16), b1c


def _pack_x(ximg):
    """[128,128,24] image -> [128, FB] haloed channel-major fp16."""
    xin = np.zeros((128, FB), np.float32)
    cols = (np.arange(-1, 129)) % WID
    for g in range(G):
        rows = (np.arange(-1, 33) + 32 * g) % H
        blk = ximg[rows][:, cols, :]
        buf = np.zeros((24, GROWS, PITCH), np.float32)
        buf[:, :, :130] = np.transpose(blk, (2, 0, 1))
        xin[32 * g : 32 * g + 24] = buf.reshape(24, FB)
    return xin.astype(np.float16)


def _pack_x8(ximg):
    """[128,128,24] image -> [128, X8L] fp8 shadow (pitch 130, wrap cols).
    Matches the device fp16->fp8 rounding closely enough (direct f32->fp8)."""
    x8 = np.zeros((128, RG, X8P), np.float32)
    cols = (np.arange(-1, 129)) % WID
    for g in range(G):
        rows = np.arange(0, 32) + 32 * g
        blk = ximg[rows][:, cols, :]  # [32, 130, 24]
        x8[32 * g : 32 * g + 24] = np.transpose(blk, (2, 0, 1))
    return x8.reshape(128, X8L).astype(np.float16).astype(F8)


def _unpack_x(xo):
    img = np.empty((H, WID, C), np.float32)
    for g in range(G):
        blk = xo[32 * g : 32 * g + 24].reshape(24, RG, WID)
        img[32 * g : 32 * g + 32] = np.transpose(blk, (1, 2, 0))
    return img


def _build_stack0(ximg, fire0):
    """Host: step-0 unified fp8 stack [128, SFL8] (srow i = image row i-1)."""
    stk0 = np.zeros((128, SROWS, SP8), np.float32)
    rows = np.arange(-1, 129) % H
    for d in range(3):
        cols = (np.arange(0, 128) + (d - 1)) % WID
        blk = ximg[rows][:, cols, :]  # [130, 128, 24]
        stk0[24 * d : 24 * d + 24] = np.transpose(blk, (2, 0, 1))
    stk0[72] = fire0[rows]
    return (
        stk0.reshape(128, SFL8).astype(np.float16).astype(F8)
    )


def _make_in_maps(x, w1, w2, W1, b1, W2, rand_u):
    apdr, apc, w2pk, b1c = _prep_weights(w1, w2, W1, b1, W2)
    in_maps = []
    for b in range(B):
        u = rand_u[:, b, :, :, 0].reshape(STEPS, H * WID)
        fire8 = np.zeros((128, 4096), F8)
        for g in range(G):
            for s in range(STEPS):
                fire8[32 * g + s] = (
                    u[s, g * 4096 : (g + 1) * 4096] < 0.5
                ).astype(F8)
        ximg = np.asarray(x[b], np.float32)
        fire0 = (u[0].reshape(H, WID) < 0.5).astype(np.float32)
        in_maps.append(
            {
                "apdr": apdr,
                "apc": apc,
                "w2p": w2pk,
                "b1col": b1c,
                "x16in": _pack_x(ximg),
                "x8in": _pack_x8(ximg),
                "firein": fire8,
                "stkin": _build_stack0(ximg, fire0),
            }
        )
    return in_maps


def kernel(x, w1, w2, W1, b1, W2, rand_u, steps, **kw):
    from concourse.bass_utils import run_bass_kernel_spmd

    assert int(steps) == STEPS
    x = np.asarray(x, np.float32)
    in_maps = _make_in_maps(
        x,
        np.asarray(w1, np.float32),
        np.asarray(w2, np.float32),
        np.asarray(W1, np.float32),
        np.asarray(b1, np.float32),
        np.asarray(W2, np.float32),
        np.asarray(rand_u, np.float32),
    )
    nc = _get_module()
    res = run_bass_kernel_spmd(nc, in_maps, core_ids=list(range(B)))
    _CACHE["last_results"] = res
    out = np.empty((B, H, WID, C), np.float32)
    for b in range(B):
        out[b] = _unpack_x(res.results[b]["xout"])
    return out


# revision 17
# speedup vs baseline: 1.1304x; 1.1304x over previous
"""Trainium2 Bass kernel for nn_BasicNCAModel — fp8 DoubleRow mm1 variant.

Same structure as the f16 kernel (unified stack, carried mm2, fire trick)
with mm1 in fp8e4: the dy=-1 and dy=+1 taps fuse into ONE DoubleRow matmul
(virtual K=256, 2 multiplies/cycle), the center tap is a normal fp8 matmul,
so mm1 is 2 matmuls per (j,g) instead of 3. The stack is fp8 pitch-128
(DoubleRow needs a single-stride moving AP: k-tile pair stride 256 B), fed
from a pitch-130 fp8 shadow x8 whose wrap cols give the dx=+-1 shifts their
circular reads. Weights are scaled by SA=32 to sit in fp8e4's normal range
(fire weight 240 = max normal; effective M = 240/32 = 7.5 >> |h+b1|), and
the 1/SA is folded into W2 (f16), so no extra scaling ops anywhere.
"""

import sys

if "/opt/trn_rl_repo" not in sys.path:
    sys.path.insert(0, "/opt/trn_rl_repo")

import numpy as np
import ml_dtypes

F8 = ml_dtypes.float8_e4m3

C = 24
NIC = 4
H = 128
WID = 128
HID = 128
STEPS = 8
B = 8
G = 4
RG = 32
PITCH = 132    # f16 master pitch
GROWS = 34
FB = GROWS * PITCH
SP8 = 128      # fp8 stack pitch (contiguous pixel rows)
SROWS = 130
SFL8 = SROWS * SP8
X8P = 130      # fp8 shadow pitch (wrap col + 128 + wrap col)
X8L = 32 * X8P
TW = 512
SA = 32.0      # fp8 weight scale
MF8 = 240.0    # fire weight (fp8e4 max normal); effective M = MF8/SA
SC_FD = 352

_CACHE = {}


def _build_module():
    from concourse import bacc, mybir, tile
    from concourse.ap import AP

    f32 = mybir.dt.float32
    f16 = mybir.dt.float16
    f8 = mybir.dt.float8e4
    Alu = mybir.AluOpType
    Act = mybir.ActivationFunctionType
    DR = mybir.MatmulPerfMode.DoubleRow

    nc = bacc.Bacc(
        "TRN2",
        target_bir_lowering=False,
        debug=False,
        enable_asserts=False,
        num_devices=8,
    )

    apdr = nc.dram_tensor("apdr", [128, 256], f8, kind="ExternalInput").ap()
    apc = nc.dram_tensor("apc", [128, 128], f8, kind="ExternalInput").ap()
    w2p = nc.dram_tensor("w2p", [128, 32], f16, kind="ExternalInput").ap()
    b1col = nc.dram_tensor("b1col", [128, 1], f32, kind="ExternalInput").ap()
    x16in = nc.dram_tensor("x16in", [128, FB], f16, kind="ExternalInput").ap()
    x8in = nc.dram_tensor("x8in", [128, X8L], f8, kind="ExternalInput").ap()
    firein = nc.dram_tensor("firein", [128, 4096], f8, kind="ExternalInput").ap()
    stkin = nc.dram_tensor("stkin", [128, SFL8], f8, kind="ExternalInput").ap()
    xout = nc.dram_tensor("xout", [128, 4096], f32, kind="ExternalOutput").ap()

    with tile.TileContext(nc) as tc:
        import contextlib

        with contextlib.ExitStack() as ctx:
            sing = ctx.enter_context(tc.tile_pool(name="sing", bufs=1))
            hpool = ctx.enter_context(tc.tile_pool(name="h", bufs=6, space="PSUM"))
            dxpool = ctx.enter_context(tc.tile_pool(name="dx", bufs=2, space="PSUM"))
            hsb = ctx.enter_context(tc.tile_pool(name="hsb", bufs=8))

            x16a = sing.tile([128, FB], f16)
            x16b = sing.tile([128, FB], f16)
            x8 = sing.tile([128, X8L], f8)
            xof = sing.tile([128, 4096], f32)
            fire = sing.tile([128, 4096], f8)
            A8 = sing.tile([128, 256], f8)
            AC = sing.tile([128, 128], f8)
            W2s = sing.tile([128, 32], f16)
            zeros = sing.tile([128, TW - SC_FD], f32)
            b1c = sing.tile([128, 1], f32)
            stk = [sing.tile([128, SFL8], f8, name=f"stk_{b}") for b in range(2)]

            nc.scalar.dma_start(A8[:], apdr[:])
            nc.scalar.dma_start(AC[:], apc[:])
            QS = [nc.sync, nc.gpsimd, nc.sync, nc.scalar]
            for g in range(G):
                lo, hi = (32 * g + 4) * SP8, (32 * g + 22) * SP8
                QS[g].dma_start(stk[0][:, lo:hi], stkin[:, lo:hi])
            gaps = [(0, 4), (22, 36), (54, 68), (86, 100), (118, 130)]
            for k, (a, b) in enumerate(gaps):
                QS[k % 3].dma_start(
                    stk[0][:, a * SP8 : b * SP8],
                    stkin[:, a * SP8 : b * SP8],
                )
            nc.gpsimd.dma_start(W2s[:], w2p[:])
            nc.gpsimd.dma_start(b1c[:], b1col[:])
            nc.sync.dma_start(x16a[:], x16in[:])
            nc.sync.dma_start(x8[:], x8in[:])
            nc.sync.dma_start(fire[:], firein[:])
            nc.gpsimd.memset(stk[1][64:128, :], 0.0)
            nc.gpsimd.memset(zeros[:], 0.0)

            xf16 = [x16a, x16b]
            fire3 = fire[:].rearrange("p (r w) -> p r w", w=128)
            xo3 = xof[:].rearrange("p (r w) -> p r w", w=128)
            x8v = x8[:].rearrange("p (r w) -> p r w", w=X8P)
            a8v = A8[:].rearrange("p (t n) -> p t n", t=2)
            SLICES = [(0, 4), (4, 16), (16, 24), (24, 32)]
            JORD = [2, 3, 1, 4, 5, 0, 6, 7]
            TRIGGER = {0: 0, 1: 1, 5: 2, 7: 3}

            def emit_slice(s, sl):
                sb = stk[(s + 1) % 2]
                la, lb = SLICES[sl]
                for g in range(G):
                    dlo = (32 * g + 1 + la) * SP8
                    dhi = (32 * g + 1 + lb) * SP8
                    for d in range(3):
                        eng = [nc.sync, nc.gpsimd][(g + d) % 2]
                        eng.dma_start(
                            sb[24 * d : 24 * d + 24, dlo:dhi],
                            x8v[32 * g : 32 * g + 24, la:lb, d : d + 128],
                        )

            def emit_fire(s):
                sb = (s + 1) % 2
                for g in range(G):
                    nc.gpsimd.dma_start(
                        stk[sb][72:73, (32 * g + 1) * SP8 : (32 * g + 33) * SP8],
                        fire3[32 * g + s + 1 : 32 * g + s + 2, :, :],
                    )

            state = {"prev": None}

            def mm2_update(s, j, hss, last):
                r0 = 4 * j + 1
                xc = xf16[s % 2][:].rearrange("p (r w) -> p r w", w=PITCH)
                xn = xf16[(s + 1) % 2][:].rearrange("p (r w) -> p r w", w=PITCH)
                dxt = dxpool.tile([128, TW], f32, tag="dx", name=f"dx_{s}_{j}")
                for g in range(G):
                    nc.tensor.matmul(
                        dxt[32 * g : 32 * g + 32, :],
                        W2s[:],
                        hss[g][:],
                        start=True,
                        stop=True,
                        tile_position=(0, 32 * g),
                    )
                dx3 = dxt[:].rearrange("p (a b) -> p a b", b=128)
                if last:
                    nc.vector.tensor_tensor(
                        xo3[:, 4 * j : 4 * j + 4, :],
                        dx3,
                        xc[:, r0 : r0 + 4, 1:129],
                        Alu.add,
                    )
                    return
                nc.vector.tensor_tensor(
                    xn[:, r0 : r0 + 4, 1:129],
                    dx3,
                    xc[:, r0 : r0 + 4, 1:129],
                    Alu.add,
                )
                nc.vector.tensor_copy(
                    xn[:, r0 : r0 + 4, 0:1], xn[:, r0 : r0 + 4, 128:129]
                )
                nc.vector.tensor_copy(
                    xn[:, r0 : r0 + 4, 129:130], xn[:, r0 : r0 + 4, 1:2]
                )
                # fp8 shadow of the updated rows (feeds next stack build)
                nc.vector.tensor_copy(
                    x8v[:, 4 * j : 4 * j + 4, 0:130],
                    xn[:, r0 : r0 + 4, 0:130],
                )
                if j in TRIGGER:
                    emit_slice(s, TRIGGER[j])
                    nb = stk[(s + 1) % 2]
                    if TRIGGER[j] == 0:
                        # halo srow 129 (image row 0) <- srow 1
                        nc.gpsimd.tensor_copy(
                            nb[:80, 129 * SP8 : 130 * SP8],
                            nb[:80, 1 * SP8 : 2 * SP8],
                        )
                    if TRIGGER[j] == 3:
                        # halo srow 0 (image row 127) <- srow 128
                        nc.gpsimd.tensor_copy(
                            nb[:80, 0:SP8],
                            nb[:80, 128 * SP8 : 129 * SP8],
                        )

            for s in range(STEPS):
                last = s + 1 == STEPS
                if not last:
                    emit_fire(s)
                sT = stk[s % 2][:]
                for j in JORD:
                    hss = []
                    for g in range(G):
                        sr0 = 32 * g + 4 * j + 1
                        ht = hpool.tile(
                            [128, TW], f32, tag="h", name=f"h_{s}_{j}_{g}"
                        )
                        # DoubleRow: k-tile 0 = dy=-1 rows, k-tile 1 =
                        # dy=+1 rows (pair stride 2 srows = 256 B)
                        rhs_dr = AP(
                            tensor=sT.tensor,
                            offset=sT.offset + (sr0 - 1) * SP8,
                            ap=[[SFL8, 128], [2 * SP8, 2], [1, 512]],
                        )
                        nc.tensor.matmul(
                            ht[:, :],
                            a8v,
                            rhs_dr,
                            start=True,
                            stop=False,
                            perf_mode=DR,
                            tile_position=(0, 0),
                        )
                        rhs_c = sT[:, sr0 * SP8 : sr0 * SP8 + TW]
                        nc.tensor.matmul(
                            ht[:, :],
                            AC[:],
                            rhs_c,
                            start=False,
                            stop=True,
                            tile_position=(0, 0),
                        )
                        hs = hsb.tile(
                            [128, TW], f16, tag="hsb", name=f"hs_{s}_{j}_{g}"
                        )
                        nc.scalar.activation(
                            hs[:, :SC_FD],
                            ht[:, :SC_FD],
                            Act.Relu,
                            bias=b1c[:],
                        )
                        nc.vector.scalar_tensor_tensor(
                            hs[:, SC_FD:],
                            ht[:, SC_FD:],
                            b1c[:],
                            zeros[:],
                            Alu.add,
                            Alu.max,
                        )
                        hss.append(hs)
                    if state["prev"] is not None:
                        mm2_update(*state["prev"])
                    state["prev"] = (s, j, hss, last)
            mm2_update(*state["prev"])

            nc.sync.dma_start(xout[:, :], xof[:, :])

    nc.compile()
    return nc


def _get_module():
    if "nc" not in _CACHE:
        _CACHE["nc"] = _build_module()
    return _CACHE["nc"]


def _prep_weights(w1, w2, W1, b1, W2):
    A = np.zeros((9, HID, C), np.float32)
    for t in range(9):
        dy, dxx = t // 3 - 1, t % 3 - 1
        A[t] = (
            W1[:, 24:48] * w1[dy + 1, dxx + 1, 0][None, :]
            + W1[:, 48:72] * w2[dy + 1, dxx + 1, 0][None, :]
        )
    A[4] += W1[:, :24]
    apdr = np.zeros((128, 256), np.float32)
    apc = np.zeros((128, 128), np.float32)
    for d in range(3):
        rows = slice(24 * d, 24 * d + 24)
        apdr[rows, 0:128] = SA * A[0 * 3 + d].T      # dy = -1
        apdr[rows, 128:256] = SA * A[2 * 3 + d].T    # dy = +1
        apc[rows, :] = SA * A[1 * 3 + d].T           # dy = 0
    apc[72, :] = MF8
    w2pk = np.zeros((128, 32), np.float32)
    w2pk[:, NIC:C] = W2[NIC:C].T / SA
    b1c = (SA * b1 - MF8).reshape(128, 1).astype(np.float32)
    return apdr.astype(F8), apc.astype(F8), w2pk.astype(np.float16), b1c


def _pack_x(ximg):
    """[128,128,24] image -> [128, FB] haloed channel-major fp16."""
    xin = np.zeros((128, FB), np.float32)
    cols = (np.arange(-1, 129)) % WID
    for g in range(G):
        rows = (np.arange(-1, 33) + 32 * g) % H
        blk = ximg[rows][:, cols, :]
        buf = np.zeros((24, GROWS, PITCH), np.float32)
        buf[:, :, :130] = np.transpose(blk, (2, 0, 1))
        xin[32 * g : 32 * g + 24] = buf.reshape(24, FB)
    return xin.astype(np.float16)


def _pack_x8(ximg):
    """[128,128,24] image -> [128, X8L] fp8 shadow (pitch 130, wrap cols).
    Matches the device fp16->fp8 rounding closely enough (direct f32->fp8)."""
    x8 = np.zeros((128, RG, X8P), np.float32)
    cols = (np.arange(-1, 129)) % WID
    for g in range(G):
        rows = np.arange(0, 32) + 32 * g
        blk = ximg[rows][:, cols, :]  # [32, 130, 24]
        x8[32 * g : 32 * g + 24] = np.transpose(blk, (2, 0, 1))
    return x8.reshape(128, X8L).astype(np.float16).astype(F8)


def _unpack_x(xo):
    img = np.empty((H, WID, C), np.float32)
    for g in range(G):
        blk = xo[32 * g : 32 * g + 24].reshape(24, RG, WID)
        img[32 * g : 32 * g + 32] = np.transpose(blk, (1, 2, 0))
    return img


def _build_stack0(ximg, fire0):
    """Host: step-0 unified fp8 stack [128, SFL8] (srow i = image row i-1)."""
    stk0 = np.zeros((128, SROWS, SP8), np.float32)
    rows = np.arange(-1, 129) % H
    for d in range(3):
        cols = (np.arange(0, 128) + (d - 1)) % WID
        blk = ximg[rows][:, cols, :]  # [130, 128, 24]
        stk0[24 * d : 24 * d + 24] = np.transpose(blk, (2, 0, 1))
    stk0[72] = fire0[rows]
    return (
        stk0.reshape(128, SFL8).astype(np.float16).astype(F8)
    )


def _make_in_maps(x, w1, w2, W1, b1, W2, rand_u):
    apdr, apc, w2pk, b1c = _prep_weights(w1, w2, W1, b1, W2)
    in_maps = []
    for b in range(B):
        u = rand_u[:, b, :, :, 0].reshape(STEPS, H * WID)
        fire8 = np.zeros((128, 4096), F8)
        for g in range(G):
            for s in range(STEPS):
                fire8[32 * g + s] = (
                    u[s, g * 4096 : (g + 1) * 4096] < 0.5
                ).astype(F8)
        ximg = np.asarray(x[b], np.float32)
        fire0 = (u[0].reshape(H, WID) < 0.5).astype(np.float32)
        in_maps.append(
            {
                "apdr": apdr,
                "apc": apc,
                "w2p": w2pk,
                "b1col": b1c,
                "x16in": _pack_x(ximg),
                "x8in": _pack_x8(ximg),
                "firein": fire8,
                "stkin": _build_stack0(ximg, fire0),
            }
        )
    return in_maps


def kernel(x, w1, w2, W1, b1, W2, rand_u, steps, **kw):
    from concourse.bass_utils import run_bass_kernel_spmd

    assert int(steps) == STEPS
    x = np.asarray(x, np.float32)
    in_maps = _make_in_maps(
        x,
        np.asarray(w1, np.float32),
        np.asarray(w2, np.float32),
        np.asarray(W1, np.float32),
        np.asarray(b1, np.float32),
        np.asarray(W2, np.float32),
        np.asarray(rand_u, np.float32),
    )
    nc = _get_module()
    res = run_bass_kernel_spmd(nc, in_maps, core_ids=list(range(B)))
    _CACHE["last_results"] = res
    out = np.empty((B, H, WID, C), np.float32)
    for b in range(B):
        out[b] = _unpack_x(res.results[b]["xout"])
    return out


# revision 19
# speedup vs baseline: 1.1722x; 1.0370x over previous
"""Trainium2 Bass kernel for nn_BasicNCAModel — fp8 DoubleRow mm1 variant.

Same structure as the f16 kernel (unified stack, carried mm2, fire trick)
with mm1 in fp8e4: the dy=-1 and dy=+1 taps fuse into ONE DoubleRow matmul
(virtual K=256, 2 multiplies/cycle), the center tap is a normal fp8 matmul,
so mm1 is 2 matmuls per (j,g) instead of 3. The stack is fp8 pitch-128
(DoubleRow needs a single-stride moving AP: k-tile pair stride 256 B), fed
from a pitch-130 fp8 shadow x8 whose wrap cols give the dx=+-1 shifts their
circular reads. Weights are scaled by SA=32 to sit in fp8e4's normal range
(fire weight 240 = max normal; effective M = 240/32 = 7.5 >> |h+b1|), and
the 1/SA is folded into W2 (f16), so no extra scaling ops anywhere.
"""

import sys

if "/opt/trn_rl_repo" not in sys.path:
    sys.path.insert(0, "/opt/trn_rl_repo")

import numpy as np
import ml_dtypes

F8 = ml_dtypes.float8_e4m3

C = 24
NIC = 4
H = 128
WID = 128
HID = 128
STEPS = 8
B = 8
G = 4
RG = 32
PITCH = 132    # f16 master pitch
GROWS = 34
FB = GROWS * PITCH
SP8 = 128      # fp8 stack pitch (contiguous pixel rows)
SROWS = 130
SFL8 = SROWS * SP8
X8P = 130      # fp8 shadow pitch (wrap col + 128 + wrap col)
X8L = 32 * X8P
TW = 512
SA = 32.0      # fp8 weight scale
MF8 = 240.0    # fire weight (fp8e4 max normal); effective M = MF8/SA
SC_FD = 352

_CACHE = {}


def _build_module():
    from concourse import bacc, mybir, tile
    from concourse.ap import AP

    f32 = mybir.dt.float32
    f16 = mybir.dt.float16
    f8 = mybir.dt.float8e4
    Alu = mybir.AluOpType
    Act = mybir.ActivationFunctionType
    DR = mybir.MatmulPerfMode.DoubleRow

    nc = bacc.Bacc(
        "TRN2",
        target_bir_lowering=False,
        debug=False,
        enable_asserts=False,
        num_devices=8,
    )

    apdr = nc.dram_tensor("apdr", [128, 256], f8, kind="ExternalInput").ap()
    apc = nc.dram_tensor("apc", [128, 128], f8, kind="ExternalInput").ap()
    w2p = nc.dram_tensor("w2p", [128, 32], f16, kind="ExternalInput").ap()
    b1col = nc.dram_tensor("b1col", [128, 1], f32, kind="ExternalInput").ap()
    x16in = nc.dram_tensor("x16in", [128, FB], f16, kind="ExternalInput").ap()
    x8in = nc.dram_tensor("x8in", [128, X8L], f8, kind="ExternalInput").ap()
    firein = nc.dram_tensor("firein", [128, 4096], f8, kind="ExternalInput").ap()
    stkin = nc.dram_tensor("stkin", [128, SFL8], f8, kind="ExternalInput").ap()
    xout = nc.dram_tensor("xout", [128, 4096], f32, kind="ExternalOutput").ap()

    with tile.TileContext(nc) as tc:
        import contextlib

        with contextlib.ExitStack() as ctx:
            sing = ctx.enter_context(tc.tile_pool(name="sing", bufs=1))
            hpool = ctx.enter_context(tc.tile_pool(name="h", bufs=6, space="PSUM"))
            dxpool = ctx.enter_context(tc.tile_pool(name="dx", bufs=2, space="PSUM"))
            hsb = ctx.enter_context(tc.tile_pool(name="hsb", bufs=8))

            x16a = sing.tile([128, FB], f16)
            x16b = sing.tile([128, FB], f16)
            x8 = sing.tile([128, X8L], f8)
            xof = sing.tile([128, 4096], f32)
            fire = sing.tile([128, 4096], f8)
            A8 = sing.tile([128, 256], f8)
            AC = sing.tile([128, 128], f8)
            W2s = sing.tile([128, 32], f16)
            zeros = sing.tile([128, TW - SC_FD], f32)
            b1c = sing.tile([128, 1], f32)
            stk = [sing.tile([128, SFL8], f8, name=f"stk_{b}") for b in range(2)]

            nc.scalar.dma_start(A8[:], apdr[:])
            nc.scalar.dma_start(AC[:], apc[:])
            QS = [nc.sync, nc.gpsimd, nc.sync, nc.scalar]
            for g in range(G):
                lo, hi = (32 * g + 4) * SP8, (32 * g + 22) * SP8
                QS[g].dma_start(stk[0][:, lo:hi], stkin[:, lo:hi])
            gaps = [(0, 4), (22, 36), (54, 68), (86, 100), (118, 130)]
            for k, (a, b) in enumerate(gaps):
                QS[k % 3].dma_start(
                    stk[0][:, a * SP8 : b * SP8],
                    stkin[:, a * SP8 : b * SP8],
                )
            nc.gpsimd.dma_start(W2s[:], w2p[:])
            nc.gpsimd.dma_start(b1c[:], b1col[:])
            nc.sync.dma_start(x16a[:], x16in[:])
            nc.sync.dma_start(x8[:], x8in[:])
            nc.sync.dma_start(fire[:], firein[:])
            nc.gpsimd.memset(stk[1][64:128, :], 0.0)
            nc.gpsimd.memset(zeros[:], 0.0)

            xf16 = [x16a, x16b]
            fire3 = fire[:].rearrange("p (r w) -> p r w", w=128)
            xo3 = xof[:].rearrange("p (r w) -> p r w", w=128)
            x8v = x8[:].rearrange("p (r w) -> p r w", w=X8P)
            a8v = A8[:].rearrange("p (t n) -> p t n", t=2)
            SLICES = [(0, 4), (4, 16), (16, 24), (24, 32)]
            JORD = [2, 3, 1, 4, 5, 0, 6, 7]
            TRIGGER = {0: 0, 1: 1, 5: 2, 7: 3}

            def emit_slice(s, sl):
                sb = stk[(s + 1) % 2]
                la, lb = SLICES[sl]
                for g in range(G):
                    dlo = (32 * g + 1 + la) * SP8
                    dhi = (32 * g + 1 + lb) * SP8
                    for d in range(3):
                        eng = [nc.sync, nc.gpsimd][(g + d) % 2]
                        eng.dma_start(
                            sb[24 * d : 24 * d + 24, dlo:dhi],
                            x8v[32 * g : 32 * g + 24, la:lb, d : d + 128],
                        )

            def emit_fire(s):
                sb = (s + 1) % 2
                for g in range(G):
                    nc.gpsimd.dma_start(
                        stk[sb][72:73, (32 * g + 1) * SP8 : (32 * g + 33) * SP8],
                        fire3[32 * g + s + 1 : 32 * g + s + 2, :, :],
                    )

            state = {"prev": None}

            def mm2_update(s, j, hss, last):
                r0 = 4 * j + 1
                xc = xf16[s % 2][:].rearrange("p (r w) -> p r w", w=PITCH)
                xn = xf16[(s + 1) % 2][:].rearrange("p (r w) -> p r w", w=PITCH)
                dxt = dxpool.tile([128, TW], f32, tag="dx", name=f"dx_{s}_{j}")
                for g in range(G):
                    nc.tensor.matmul(
                        dxt[32 * g : 32 * g + 32, :],
                        W2s[:],
                        hss[g][:],
                        start=True,
                        stop=True,
                        tile_position=(0, 32 * g),
                    )
                dx3 = dxt[:].rearrange("p (a b) -> p a b", b=128)
                if last:
                    nc.vector.tensor_tensor(
                        xo3[:, 4 * j : 4 * j + 4, :],
                        dx3,
                        xc[:, r0 : r0 + 4, 1:129],
                        Alu.add,
                    )
                    # stream this tile's rows out now (overlaps the rest
                    # of the final step instead of one big tail DMA)
                    nc.sync.dma_start(
                        xout[:, 512 * j : 512 * j + 512],
                        xof[:, 512 * j : 512 * j + 512],
                    )
                    return
                nc.vector.tensor_tensor(
                    xn[:, r0 : r0 + 4, 1:129],
                    dx3,
                    xc[:, r0 : r0 + 4, 1:129],
                    Alu.add,
                )
                nc.vector.tensor_copy(
                    xn[:, r0 : r0 + 4, 0:1], xn[:, r0 : r0 + 4, 128:129]
                )
                nc.vector.tensor_copy(
                    xn[:, r0 : r0 + 4, 129:130], xn[:, r0 : r0 + 4, 1:2]
                )
                # fp8 shadow of the updated rows (feeds next stack build)
                nc.vector.tensor_copy(
                    x8v[:, 4 * j : 4 * j + 4, 0:130],
                    xn[:, r0 : r0 + 4, 0:130],
                )
                if j in TRIGGER:
                    emit_slice(s, TRIGGER[j])
                    nb = stk[(s + 1) % 2]
                    if TRIGGER[j] == 0:
                        # halo srow 129 (image row 0) <- srow 1
                        nc.gpsimd.tensor_copy(
                            nb[:80, 129 * SP8 : 130 * SP8],
                            nb[:80, 1 * SP8 : 2 * SP8],
                        )
                    if TRIGGER[j] == 3:
                        # halo srow 0 (image row 127) <- srow 128
                        nc.gpsimd.tensor_copy(
                            nb[:80, 0:SP8],
                            nb[:80, 128 * SP8 : 129 * SP8],
                        )

            for s in range(STEPS):
                last = s + 1 == STEPS
                if not last:
                    emit_fire(s)
                sT = stk[s % 2][:]
                for j in JORD:
                    hss = []
                    for g in range(G):
                        sr0 = 32 * g + 4 * j + 1
                        ht = hpool.tile(
                            [128, TW], f32, tag="h", name=f"h_{s}_{j}_{g}"
                        )
                        # DoubleRow: k-tile 0 = dy=-1 rows, k-tile 1 =
                        # dy=+1 rows (pair stride 2 srows = 256 B)
                        rhs_dr = AP(
                            tensor=sT.tensor,
                            offset=sT.offset + (sr0 - 1) * SP8,
                            ap=[[SFL8, 128], [2 * SP8, 2], [1, 512]],
                        )
                        nc.tensor.matmul(
                            ht[:, :],
                            a8v,
                            rhs_dr,
                            start=True,
                            stop=False,
                            perf_mode=DR,
                            tile_position=(0, 0),
                        )
                        rhs_c = sT[:, sr0 * SP8 : sr0 * SP8 + TW]
                        nc.tensor.matmul(
                            ht[:, :],
                            AC[:],
                            rhs_c,
                            start=False,
                            stop=True,
                            tile_position=(0, 0),
                        )
                        hs = hsb.tile(
                            [128, TW], f16, tag="hsb", name=f"hs_{s}_{j}_{g}"
                        )
                        nc.scalar.activation(
                            hs[:, :SC_FD],
                            ht[:, :SC_FD],
                            Act.Relu,
                            bias=b1c[:],
                        )
                        nc.vector.scalar_tensor_tensor(
                            hs[:, SC_FD:],
                            ht[:, SC_FD:],
                            b1c[:],
                            zeros[:],
                            Alu.add,
                            Alu.max,
                        )
                        hss.append(hs)
                    if state["prev"] is not None:
                        mm2_update(*state["prev"])
                    state["prev"] = (s, j, hss, last)
            mm2_update(*state["prev"])

    nc.compile()
    return nc


def _get_module():
    if "nc" not in _CACHE:
        _CACHE["nc"] = _build_module()
    return _CACHE["nc"]


def _prep_weights(w1, w2, W1, b1, W2):
    A = np.zeros((9, HID, C), np.float32)
    for t in range(9):
        dy, dxx = t // 3 - 1, t % 3 - 1
        A[t] = (
            W1[:, 24:48] * w1[dy + 1, dxx + 1, 0][None, :]
            + W1[:, 48:72] * w2[dy + 1, dxx + 1, 0][None, :]
        )
    A[4] += W1[:, :24]
    apdr = np.zeros((128, 256), np.float32)
    apc = np.zeros((128, 128), np.float32)
    for d in range(3):
        rows = slice(24 * d, 24 * d + 24)
        apdr[rows, 0:128] = SA * A[0 * 3 + d].T      # dy = -1
        apdr[rows, 128:256] = SA * A[2 * 3 + d].T    # dy = +1
        apc[rows, :] = SA * A[1 * 3 + d].T           # dy = 0
    apc[72, :] = MF8
    w2pk = np.zeros((128, 32), np.float32)
    w2pk[:, NIC:C] = W2[NIC:C].T / SA
    b1c = (SA * b1 - MF8).reshape(128, 1).astype(np.float32)
    return apdr.astype(F8), apc.astype(F8), w2pk.astype(np.float16), b1c


def _pack_x(ximg):
    """[128,128,24] image -> [128, FB] haloed channel-major fp16."""
    xin = np.zeros((128, FB), np.float32)
    cols = (np.arange(-1, 129)) % WID
    for g in range(G):
        rows = (np.arange(-1, 33) + 32 * g) % H
        blk = ximg[rows][:, cols, :]
        buf = np.zeros((24, GROWS, PITCH), np.float32)
        buf[:, :, :130] = np.transpose(blk, (2, 0, 1))
        xin[32 * g : 32 * g + 24] = buf.reshape(24, FB)
    return xin.astype(np.float16)


def _pack_x8(ximg):
    """[128,128,24] image -> [128, X8L] fp8 shadow (pitch 130, wrap cols).
    Matches the device fp16->fp8 rounding closely enough (direct f32->fp8)."""
    x8 = np.zeros((128, RG, X8P), np.float32)
    cols = (np.arange(-1, 129)) % WID
    for g in range(G):
        rows = np.arange(0, 32) + 32 * g
        blk = ximg[rows][:, cols, :]  # [32, 130, 24]
        x8[32 * g : 32 * g + 24] = np.transpose(blk, (2, 0, 1))
    return x8.reshape(128, X8L).astype(np.float16).astype(F8)


def _unpack_x(xo):
    img = np.empty((H, WID, C), np.float32)
    for g in range(G):
        blk = xo[32 * g : 32 * g + 24].reshape(24, RG, WID)
        img[32 * g : 32 * g + 32] = np.transpose(blk, (1, 2, 0))
    return img


def _build_stack0(ximg, fire0):
    """Host: step-0 unified fp8 stack [128, SFL8] (srow i = image row i-1)."""
    stk0 = np.zeros((128, SROWS, SP8), np.float32)
    rows = np.arange(-1, 129) % H
    for d in range(3):
        cols = (np.arange(0, 128) + (d - 1)) % WID
        blk = ximg[rows][:, cols, :]  # [130, 128, 24]
        stk0[24 * d : 24 * d + 24] = np.transpose(blk, (2, 0, 1))
    stk0[72] = fire0[rows]
    return (
        stk0.reshape(128, SFL8).astype(np.float16).astype(F8)
    )


def _make_in_maps(x, w1, w2, W1, b1, W2, rand_u):
    apdr, apc, w2pk, b1c = _prep_weights(w1, w2, W1, b1, W2)
    in_maps = []
    for b in range(B):
        u = rand_u[:, b, :, :, 0].reshape(STEPS, H * WID)
        fire8 = np.zeros((128, 4096), F8)
        for g in range(G):
            for s in range(STEPS):
                fire8[32 * g + s] = (
                    u[s, g * 4096 : (g + 1) * 4096] < 0.5
                ).astype(F8)
        ximg = np.asarray(x[b], np.float32)
        fire0 = (u[0].reshape(H, WID) < 0.5).astype(np.float32)
        in_maps.append(
            {
                "apdr": apdr,
                "apc": apc,
                "w2p": w2pk,
                "b1col": b1c,
                "x16in": _pack_x(ximg),
                "x8in": _pack_x8(ximg),
                "firein": fire8,
                "stkin": _build_stack0(ximg, fire0),
            }
        )
    return in_maps


def kernel(x, w1, w2, W1, b1, W2, rand_u, steps, **kw):
    from concourse.bass_utils import run_bass_kernel_spmd

    assert int(steps) == STEPS
    x = np.asarray(x, np.float32)
    in_maps = _make_in_maps(
        x,
        np.asarray(w1, np.float32),
        np.asarray(w2, np.float32),
        np.asarray(W1, np.float32),
        np.asarray(b1, np.float32),
        np.asarray(W2, np.float32),
        np.asarray(rand_u, np.float32),
    )
    nc = _get_module()
    res = run_bass_kernel_spmd(nc, in_maps, core_ids=list(range(B)))
    _CACHE["last_results"] = res
    out = np.empty((B, H, WID, C), np.float32)
    for b in range(B):
        out[b] = _unpack_x(res.results[b]["xout"])
    return out
